# revision 58
# baseline (speedup 1.0000x reference)
"""GAT layer (nn_GATLayer_44220983279640) — Trainium2 Bass/Tile kernel, v2.

Reference math per graph (B=16, D=512, FIN=FOUT=128, H=8):
    h  = x @ W                                         [D, F]
    s1[hd,i] = h[i] . a1[hd]   s2b[hd,j] = h[j] . a2[hd] + ab[hd]
    e  = leaky_relu(s1[:,None] + s2b[None,:])          [H, D, D]
    att = softmax_j(where(adj > 0, e, -9e15))
    out = mean_hd(att @ h)                             [D, F]

Sharding: data-parallel over batch, 2 graphs per core on 8 cores.

Exact softmax reformulation (rows rescaled by exp(-(s1_i + 2))):
    E'[j,i] = adj[j,i] * max(B_j, C_i * D_j)
    B_j = exp(s2b_j - 2)      C_i = exp(-0.99 s1_i - 2)      D_j = exp(0.01 s2b_j)

v2: the whole E' construction is ONE fused custom DVE op per j-chunk:
    out = max(in0*s0, s1) * in1   (in0=C bcast, in1=mask, s0=D_j, s1=B_j)
with a hand-authored 2x_1p uop variant (2 fp16 elems/lane/cycle), replacing
the v1 chain of 4 tensor_scalar + 1 tensor_tensor per head. Steady-state
DVE work per head-graph drops ~2.8us -> ~1.6us.

Other v2 changes vs the 73us baseline:
  * software-pipelined finish: recip/normalize/accumulate for head k are
    emitted after head k+1's E+matmuls, so the in-order DVE/ACT queues never
    stall on PE completion.
  * 1/H folded into the rowsum ones-column (value H) -> haug copies need no
    scale -> graph 1's haug copy runs on idle GPSIMD, B/D exps merged to 2
    ACT ops (one [P,32] Exp each), h copies merged to 1.
  * head-0 C broadcast via a K=1 PE matmul (ones row x c_sb row) instead of
    the DRAM staging round trip; heads 1-7 still use staged stride-0 DMA.
  * output DMA reads the accumulator PSUM directly (no ACT copy).
  * x DMAs issued first; masks paced behind them on the other queue.
"""

from contextlib import ExitStack

import numpy as np

import concourse.bass as bass
import concourse.bacc as bacc
import concourse.tile as tile
from concourse import mybir
from concourse.bass_utils import run_bass_kernel_spmd

import concourse.dve_ops as dve_ops
from concourse.dve_spec import Spec, Src0, Src1, C0, C1, maxx, lower
from concourse.dve_uop import (
    DveOpSpec, UopConfig, InpSel, OutSel, OutPath,
    AluOp, AluInp, DelayInp, Trigger, ENABLE,
)

B, D, FIN, FOUT, H = 16, 512, 128, 128, 8
NCORES = 8
NB = B // NCORES          # graphs per core
P = 128                   # partitions
NCH = D // P              # 4 j-chunks / i-tiles
DELTA = -2.0              # global exp downshift (cancels in softmax)

EMASK_MODE = "2x"         # "2x" | "1x" (custom op without perf slot)

F32 = mybir.dt.float32
F16 = mybir.dt.float16

# consts f32 [P, 2]: delta col | C-bias col (-0.99*ab + DELTA)
CDL = 0
CBC = 1
CONST_COLS = 2

# constsH fp16 [P, 144]: W | Wa1 | Wa2
CW0, CW1 = 0, FOUT
CA1 = CW1 + H                           # Wa1 = W @ a1^T  [FIN, H]
CA2 = CA1 + H                           # Wa2 = W @ a2^T  [FIN, H]
CONSTH_COLS = CA2 + H

_NC_CACHE = {}
_EMASK = {}


# --------------------------------------------------------------------------
# custom DVE op: out = max(in0 * s0, s1) * in1, with 2x_1p table slot
# --------------------------------------------------------------------------

def _emask_ref(in0, in1, s0, s1, imm2):
    return np.maximum(in0.astype(np.float32) * s0, s1) * in1.astype(np.float32)


def _build_2x_uop():
    """2x_1p program: per cycle two packed fp16 elems (lo/hi) flow through
    6 compute blocks (mult, max, mult for each half) + 2 shuttle blocks.
    Mirrors the stock 2X_1P idiom: WR0_LO <- ALU_OUT, WR0_HI <- DELAY_0."""
    u = UopConfig()
    u.enable_input(InpSel.SRC_0, 1)      # d0 @blk0: cb lo
    u.enable_input(InpSel.CONST_0, 2)    # d1: D scalar
    u.enable_input(InpSel.CONST_1, 3)    # d2: B scalar
    u.enable_input(InpSel.SRC_1, 4)      # d3: mask lo
    u.enable_input(InpSel.SRC_0_HI, 5)   # d4: cb hi
    u.enable_input(InpSel.SRC_1_HI, 6)   # d5: mask hi
    dp = u.datapath_config
    dp[0].enable_alu(AluOp.MULTIPLY, AluInp.PREV_DELAY_0, AluInp.PREV_DELAY_1)
    dp[0].pass_through_delay(1, 2, 3, 4, 5)
    dp[1].enable_alu(AluOp.MAX, AluInp.PREV_ALU_OUT, AluInp.PREV_DELAY_2)
    dp[1].pass_through_delay(1, 2, 3, 4, 5)
    dp[2].enable_alu(AluOp.MULTIPLY, AluInp.PREV_ALU_OUT, AluInp.PREV_DELAY_3)
    dp[2].pass_through_delay(1, 2, 4, 5)
    dp[3].enable_alu(AluOp.MULTIPLY, AluInp.PREV_DELAY_4, AluInp.PREV_DELAY_1)
    dp[3].enable_delay_from_src(DelayInp.PREV_ALU_OUT, 0)   # capture LO
    dp[3].pass_through_delay(2, 5)
    dp[4].enable_alu(AluOp.MAX, AluInp.PREV_ALU_OUT, AluInp.PREV_DELAY_2)
    dp[4].pass_through_delay(0, 5)
    dp[5].enable_alu(AluOp.MULTIPLY, AluInp.PREV_ALU_OUT, AluInp.PREV_DELAY_5)
    dp[5].pass_through_delay(0)
    dp[6].enable_alu(AluOp.BYPASS, AluInp.PREV_DELAY_0, AluInp.PREV_DELAY_0)
    dp[6].enable_delay_from_src(DelayInp.PREV_ALU_OUT, 0)   # d0 <- HI
    dp[7].enable_alu(AluOp.BYPASS, AluInp.PREV_ALU_OUT, AluInp.PREV_ALU_OUT)
    dp[7].pass_through_delay(0)
    u.require_inp0 = ENABLE
    u.require_inp1 = ENABLE
    u.trigger = (Trigger.SRC_TENSOR_DONE, Trigger.NONE, Trigger.NONE)
    u.enable_output(OutSel.ALU_OUT, OutPath.WR0_LO)
    u.enable_output(OutSel.DELAY_0, OutPath.WR0_HI)
    return u


def _get_emask_op():
    perf = EMASK_MODE == "2x"
    name = "GAT_EMASK2X_ANT" if perf else "GAT_EMASK1X_ANT"
    if name in _EMASK:
        return _EMASK[name]
    spec = Spec(body=maxx(Src0 * C0, C1) * Src1, reference=_emask_ref)
    uops_1x = lower(spec, ver="v3")
    kw = dict(uops_2x=[_build_2x_uop()], perf_max=1) if perf else {}
    row = dve_ops._CUSTOM_DVE_ROW_BASE + len(dve_ops.OPS)
    spec_obj = DveOpSpec(name=name, opcode=row, uops=uops_1x, rd1_en=True, **kw)
    op = dve_ops.DveOp(name, spec, subdim=False,
                       uops_sha={"v3": spec_obj.sha("v3")})
    dve_ops.OPS.append(op)
    dve_ops.CUSTOM_DVE_SPECS[name] = spec
    dve_ops._SUB_OPCODE_FOR_NAME[name] = row
    dve_ops._COMPILE_CACHE[(name, "v3")] = spec_obj
    _EMASK[name] = op
    return op


def _emask(nc, out, in0, in1, s0, s1):
    # Inline of nc.vector._custom_dve that passes perf_max at construction
    # (add_instruction stores a copy, so post-hoc assignment is lost).
    import concourse.bass_isa as bass_isa

    op = _get_emask_op()
    v = nc.vector
    if op.name not in v.bass.m.ant_custom_dve_ops:
        v.bass.m.ant_custom_dve_ops = sorted(
            {*v.bass.m.ant_custom_dve_ops, op.name}
        )
    op.compile("v3")
    shape = bass_isa.CustomDveShape.TTSS
    isa_opcode = v.bass.isa.Opcode[
        f"NEURON_ISA_TPB_OPCODE_CUSTOM_DVE_ANT_{shape.slot()}"
    ].value
    ins = [
        v.lower_ap(in0, for_isa=True, opt=True),
        v.lower_ap(in1, for_isa=True, opt=True),
        v.lower_ap(s0, for_isa=True),
        v.lower_ap(s1, for_isa=True),
    ]
    outs = [v.lower_ap(out, for_isa=True, opt=True)]
    kw = {"perf_max": 1} if EMASK_MODE == "2x" else {}
    return v.add_instruction(
        bass_isa.InstCustomDveAnt(
            name=v.bass.get_next_instruction_name(),
            op_name=op.name, rd1_en=True, subdim=0, imm2=0.0,
            shape=shape, row=dve_ops.get_dve_sub_opcode(op.name),
            isa_opcode=isa_opcode, ins=ins, outs=outs, **kw,
        )
    )


# --------------------------------------------------------------------------
# kernel body
# --------------------------------------------------------------------------

def _build_bass():
    nc = bacc.Bacc("TRN2", debug=False, num_devices=NCORES)

    xT = nc.dram_tensor("xT", [NB, FIN, D], F16, kind="ExternalInput").ap()
    adjm = nc.dram_tensor("adjm", [NB, P, NCH * D], F16, kind="ExternalInput").ap()
    consts = nc.dram_tensor("consts", [P, CONST_COLS], F32, kind="ExternalInput").ap()
    constsH = nc.dram_tensor("constsH", [P, CONSTH_COLS], F16, kind="ExternalInput").ap()
    hsel = nc.dram_tensor("hsel", [NSEL, NSEL * P], F16, kind="ExternalInput").ap()
    ident = nc.dram_tensor("ident", [P, P], F16, kind="ExternalInput").ap()
    cd = nc.dram_tensor("cd", [NB, H, D], F16).ap()
    out = nc.dram_tensor("out", [NB, D, FOUT], F16, kind="ExternalOutput").ap()

    with tile.TileContext(nc) as tc, ExitStack() as ctx:
        _kernel_body(ctx, tc, out, xT, adjm, consts, constsH, hsel, ident, cd)
    nc.compile()
    return nc


def _kernel_body(ctx, tc, out, xT, adjm, consts, constsH, hsel, ident, cd):
    nc = tc.nc
    Copy = mybir.ActivationFunctionType.Copy
    Exp = mybir.ActivationFunctionType.Exp

    const = ctx.enter_context(tc.tile_pool(name="const", bufs=1))
    xpool = ctx.enter_context(tc.tile_pool(name="xpool", bufs=NB))
    mpool = ctx.enter_context(tc.tile_pool(name="mpool", bufs=NB))
    hpool = ctx.enter_context(tc.tile_pool(name="hpool", bufs=NB))
    bdpool = ctx.enter_context(tc.tile_pool(name="bdpool", bufs=2 * NB))
    crow = ctx.enter_context(tc.tile_pool(name="crow", bufs=NB))
    cbpool = ctx.enter_context(tc.tile_pool(name="cbpool", bufs=NB))
    epool = ctx.enter_context(tc.tile_pool(name="epool", bufs=4))
    unpool = ctx.enter_context(tc.tile_pool(name="unpool", bufs=3))
    rpool = ctx.enter_context(tc.tile_pool(name="rpool", bufs=4))
    aspool = ctx.enter_context(tc.tile_pool(name="aspool", bufs=NB))
    # PSUM: 2 setup + 4 agg (2 packed tiles x 2 bufs) + 2 accumulators = 8
    pset = ctx.enter_context(tc.tile_pool(name="pset", bufs=2, space="PSUM"))
    pout = ctx.enter_context(tc.tile_pool(name="pout", bufs=4, space="PSUM"))
    pacc = ctx.enter_context(tc.tile_pool(name="pacc", bufs=NB, space="PSUM"))

    # ---- phase A ---------------------------------------------------------
    # DGE descriptor generation costs ~0.6-1.1us PER dma_start, serially per
    # queue — so x0 (the gate for all setup compute) is the sole first issue
    # on the scalar queue; x1+masks flow on sync; consts + staged/derived
    # data ride gpsimd.
    xs0 = xpool.tile([FIN, D], F16, tag="x", name="x0")
    nc.scalar.dma_start(out=xs0, in_=xT[0])
    cst = const.tile([P, CONST_COLS], F32)
    nc.sync.dma_start(out=cst, in_=consts)
    csth = const.tile([P, CONSTH_COLS], F16)
    nc.sync.dma_start(out=csth, in_=constsH)
    xs1 = xpool.tile([FIN, D], F16, tag="x", name="x1")
    nc.sync.dma_start(out=xs1, in_=xT[1])
    x_sb = [xs0, xs1]
    m_sb = []
    for b in range(NB):
        ms = mpool.tile([P, NCH * D], F16, tag="m")
        nc.sync.dma_start(out=ms, in_=adjm[b])
        m_sb.append(ms)
    hsel_sb = const.tile([NSEL, NSEL * P], F16)
    nc.gpsimd.dma_start(out=hsel_sb, in_=hsel)
    I_sb = const.tile([P, P], F16)
    nc.gpsimd.dma_start(out=I_sb, in_=ident)
    W_sb = csth[:, CW0:CW1]
    Wa1_sb = csth[:, CW1:CA1]
    Wa2_sb = csth[:, CA1:CA2]
    def head_sel(hd):
        return hsel_sb[0:NSEL, hd * P:(hd + 1) * P]
    dl_col = cst[:, CDL:CDL + 1]
    cb_col = cst[:, CBC:CBC + 1]

    G = [dict(m_sb=m_sb[b]) for b in range(NB)]

    def setup_compute(b):
        """PE matmuls + exps for graph b. ab is folded into the C exp bias
        (cb_col), so s2 needs only fp16 matmuls. haug copies ride DVE (idle
        in lead-in). For b=0, heads 0-1's C broadcast comes from K=1 PE
        matmuls (no DRAM round trip); everything else via staged DMA on the
        gpsimd queue (the scalar queue is the busy ACT engine's sequencer)."""
        g = G[b]
        xs = x_sb[b]
        cb_all = cbpool.tile([P, H, D], F16, tag="cb")
        g["cb_all"] = cb_all

        # all x-gated matmuls first so the PE queue never stalls on ACT
        p_s1 = pset.tile([P, D], F32, tag="setup")
        nc.tensor.matmul(p_s1[0:H, :], Wa1_sb, xs[:], start=True, stop=True)
        p_s2 = pset.tile([P, NCH * H], F32, tag="setup")
        for c in range(NCH):
            nc.tensor.matmul(p_s2[:, c * H:(c + 1) * H],
                             xs[:, bass.ts(c, P)], Wa2_sb,
                             start=True, stop=True)
        p_h = pset.tile([P, NCH * FOUT], F32, tag="setup")
        for c in range(NCH):
            nc.tensor.matmul(p_h[:, c * FOUT:(c + 1) * FOUT],
                             xs[:, bass.ts(c, P)], W_sb, start=True, stop=True)

        # exps: B/D first (they gate the first E together with cb0)
        B_all = bdpool.tile([P, NCH * H], F32, tag=f"B{b}")
        nc.scalar.activation(B_all[:], p_s2[:], Exp, bias=dl_col)
        D_all = bdpool.tile([P, NCH * H], F32, tag=f"D{b}")
        nc.scalar.activation(D_all[:], p_s2[:], Exp, scale=0.01)
        g["B_all"], g["D_all"] = B_all, D_all
        c_sb = crow.tile([H, D], F16, tag="Crow")
        nc.scalar.activation(
            c_sb[:], p_s1[0:H, :], Exp, scale=-0.99, bias=cb_col[0:H, :]
        )
        nc.gpsimd.dma_start(out=cd[b], in_=c_sb[:])

        # graph 0 heads 0..NSEL-1: C broadcast via K=NSEL PE matmuls into the
        # (still idle) accumulator banks — no DRAM round-trip latency
        def pe_bcast(hd):
            p_cb = pacc.tile([P, D], F32, tag="acc", name=f"pcb{hd}")
            nc.tensor.matmul(p_cb[:], head_sel(hd), c_sb[0:NSEL, :],
                             start=True, stop=True)
            nc.scalar.activation(cb_all[:, hd, :], p_cb[:], Copy)

        # haug copy: DVE for graph 0 (idle in lead-in); ACT for graph 1 (the
        # DVE paces the steady loop by then, ACT has the slack)
        haug = hpool.tile([P, NCH, FOUT + 1], F16, tag="haug")
        hview = bass.AP(
            tensor=haug.tensor, offset=haug.offset,
            ap=[haug.ap[0], [FOUT + 1, NCH], [1, FOUT]],
        )
        if b == 0:
            nc.vector.tensor_copy(out=hview, in_=p_h[:])
        else:
            nc.scalar.activation(hview, p_h[:], Copy)
        for c in range(NCH):
            nc.vector.memset(haug[:, c, FOUT:FOUT + 1], float(H))
        g["haug"] = haug

        if b == 0:
            for hd in range(NSEL):
                pe_bcast(hd)

        # remaining heads' C broadcast via staged stride-0 DMA on the gpsimd
        # queue (the scalar queue is the busy ACT engine's sequencer)
        row0 = cd[b, 0]
        slices = [(NSEL, H)] if b == 0 else [(0, NSEL), (NSEL, H)]
        for lo, hi in slices:
            nc.gpsimd.dma_start(
                out=cb_all[:, lo:hi, :],
                in_=bass.AP(
                    tensor=cd.tensor, offset=row0.offset + lo * D,
                    ap=[[0, P], [D, hi - lo], row0.ap[-1]],
                ),
            )
        g["p_acc"] = pacc.tile([P, NCH * FOUT], F32, tag="acc", name=f"pacc{b}")

    setup_compute(0)

    # ---- main head loop, software-pipelined finish -----------------------
    def emit_E_and_agg(b, hd):
        g = G[b]
        cb = g["cb_all"][:, hd, :]
        E = epool.tile([P, NCH * D], F16, tag="E")
        for c in range(NCH):
            _emask(nc, E[:, bass.ts(c, D)], cb, g["m_sb"][:, bass.ts(c, D)],
                   g["D_all"][:, c * H + hd:c * H + hd + 1],
                   g["B_all"][:, c * H + hd:c * H + hd + 1])
        po2 = [pout.tile([P, 2, FOUT + 1], F32, tag="po", name=f"po{k}")
               for k in range(2)]
        p_os = [po2[t // 2][:, t % 2, :] for t in range(NCH)]
        for t in range(NCH):
            for c in range(NCH):
                nc.tensor.matmul(
                    p_os[t],
                    E[:, c * D + t * P: c * D + (t + 1) * P],
                    g["haug"][:, c, :],
                    start=(c == 0),
                    stop=(c == NCH - 1),
                )
        return po2, p_os

    def emit_finish(b, hd, po2, p_os):
        g = G[b]
        rall = rpool.tile([P, NCH], F32, tag="r")
        for k in range(2):
            nc.vector.reciprocal(rall[:, 2 * k:2 * k + 2], po2[k][:, :, FOUT])
        un = unpool.tile([P, NCH * FOUT], F16, tag="un")
        last = b == NB - 1 and hd == H - 1
        for t in range(NCH):
            if last and t % 2 == 1:
                # final drain: the now-idle DVE normalizes half the tiles in
                # parallel with ACT
                nc.vector.tensor_scalar_mul(
                    un[:, bass.ts(t, FOUT)], p_os[t][:, 0:FOUT],
                    rall[:, t:t + 1],
                )
            else:
                nc.scalar.activation(
                    un[:, bass.ts(t, FOUT)], p_os[t][:, 0:FOUT], Copy,
                    scale=rall[:, t:t + 1],
                )
        if hd < H - 1:
            nc.tensor.matmul(
                g["p_acc"][:], I_sb, un[:], start=(hd == 0), stop=False
            )
            return
        # last head: split accumulate/copy/store so output DMAs overlap the
        # remaining normalize+accumulate work (drain shortening). For the
        # final graph, quarter it across engines and DMA queues.
        acc_sb = aspool.tile([P, NCH * FOUT], F16, tag="accsb",
                             name=f"accsb{b}")
        ob = out[b]
        last = b == NB - 1
        nq = 2
        QW = NCH // nq
        queues = [nc.sync, nc.scalar, nc.gpsimd, nc.sync]
        for k in range(nq):
            cols = slice(k * QW * FOUT, (k + 1) * QW * FOUT)
            nc.tensor.matmul(
                g["p_acc"][:, cols], I_sb, un[:, cols], start=False, stop=True
            )
            if last and k % 2 == 1:
                nc.vector.tensor_copy(out=acc_sb[:, cols],
                                      in_=g["p_acc"][:, cols])
            else:
                nc.scalar.activation(acc_sb[:, cols], g["p_acc"][:, cols],
                                     Copy)
            (queues[k] if last else nc.sync).dma_start(
                out=bass.AP(
                    tensor=out.tensor,
                    offset=ob.offset + k * QW * P * FOUT,
                    ap=[[FOUT, P], [P * FOUT, QW], [1, FOUT]],
                ),
                in_=acc_sb[:, cols],
            )

    pend = None
    for b in range(NB):
        for hd in range(H):
            cur = (b, hd, *emit_E_and_agg(b, hd))
            if pend is not None:
                emit_finish(*pend)
            pend = cur
            if b == 0 and hd == 4:
                # graph 1 setup lands here, past the ramp: the ramp's first
                # iterations have no ACT slack for extra exps/copies.
                setup_compute(1)
    emit_finish(*pend)


# --------------------------------------------------------------------------
# host-side packing
# --------------------------------------------------------------------------

def _prep_core_inputs(input, adj, W, a_w, a_b, core):
    gs = slice(core * NB, (core + 1) * NB)
    x_c = np.asarray(input[gs], dtype=np.float32)     # [NB, D, FIN]
    adj_c = np.asarray(adj[gs])                       # [NB, D, D] int32
    xT = np.ascontiguousarray(x_c.transpose(0, 2, 1)).astype(np.float16)
    adjT = (adj_c.transpose(0, 2, 1) > 0)             # [NB, j, i]
    # [NB, j, i] -> [NB, p, c, i]  (j = c*128 + p)
    adjm = np.ascontiguousarray(
        adjT.reshape(NB, NCH, P, D).transpose(0, 2, 1, 3)
        .reshape(NB, P, NCH * D).astype(np.float16)
    )
    return {
        "xT": xT,
        "adjm": adjm,
        "consts": _pack_consts(W, a_w, a_b),
        "constsH": _pack_consts_h(W, a_w),
        "hsel": _pack_hsel(),
        "ident": np.eye(P, dtype=np.float16),
    }


def _pack_consts_h(W, a_w):
    W = np.asarray(W, dtype=np.float32)
    a_w = np.asarray(a_w, dtype=np.float32)
    c = np.zeros((P, CONSTH_COLS), dtype=np.float32)
    c[:, CW0:CW1] = W
    c[:, CW1:CA1] = W @ a_w[:, :FOUT].T               # Wa1 [FIN, H]
    c[:, CA1:CA2] = W @ a_w[:, FOUT:].T               # Wa2 [FIN, H]
    return c.astype(np.float16)


NSEL = 4  # heads broadcast via PE instead of the staged-DMA round trip


def _pack_hsel():
    # one-hot selector columns: bcast matmul lhsT [NSEL, P] picks c_sb row hd
    c = np.zeros((NSEL, NSEL * P), dtype=np.float16)
    for hd in range(NSEL):
        c[hd, hd * P:(hd + 1) * P] = 1.0
    return c


def _pack_consts(W, a_w, a_b):
    a_b = np.asarray(a_b, dtype=np.float32)
    c = np.zeros((P, CONST_COLS), dtype=np.float32)
    c[:, CDL] = DELTA
    # ab folded into the C exp: exp(ab_h) scales head h's whole E matrix and
    # cancels in the softmax row normalization, leaving a -0.99*ab_h shift on C.
    c[:H, CBC] = -0.99 * a_b + DELTA
    return c


def get_nc():
    if "nc" not in _NC_CACHE:
        _NC_CACHE["nc"] = _build_bass()
    return _NC_CACHE["nc"]


def run_on_device(in_maps, **kwargs):
    return run_bass_kernel_spmd(get_nc(), in_maps, list(range(NCORES)), **kwargs)


def kernel(input, adj, W, a_w, a_b):
    input = np.asarray(input, dtype=np.float32)
    adj = np.asarray(adj)

    in_maps = [
        _prep_core_inputs(input, adj, W, a_w, a_b, c) for c in range(NCORES)
    ]
    res = run_on_device(in_maps)
    outs = [res.results[c]["out"] for c in range(NCORES)]
    return np.concatenate(outs, axis=0).astype(np.float32)


if __name__ == "__main__":
    nc = get_nc()
    print("built ok")


# revision 59
# speedup vs baseline: 1.1910x; 1.1910x over previous
"""GAT layer (nn_GATLayer_44220983279640) — Trainium2 Bass/Tile kernel, v2.

Reference math per graph (B=16, D=512, FIN=FOUT=128, H=8):
    h  = x @ W                                         [D, F]
    s1[hd,i] = h[i] . a1[hd]   s2b[hd,j] = h[j] . a2[hd] + ab[hd]
    e  = leaky_relu(s1[:,None] + s2b[None,:])          [H, D, D]
    att = softmax_j(where(adj > 0, e, -9e15))
    out = mean_hd(att @ h)                             [D, F]

Sharding: data-parallel over batch, 2 graphs per core on 8 cores.

Exact softmax reformulation (rows rescaled by exp(-(s1_i + 2))):
    E'[j,i] = adj[j,i] * max(B_j, C_i * D_j)
    B_j = exp(s2b_j - 2)      C_i = exp(-0.99 s1_i - 2)      D_j = exp(0.01 s2b_j)

v2: the whole E' construction is ONE fused custom DVE op per j-chunk:
    out = max(in0*s0, s1) * in1   (in0=C bcast, in1=mask, s0=D_j, s1=B_j)
with a hand-authored 2x_1p uop variant (2 fp16 elems/lane/cycle), replacing
the v1 chain of 4 tensor_scalar + 1 tensor_tensor per head. Steady-state
DVE work per head-graph drops ~2.8us -> ~1.6us.

Other v2 changes vs the 73us baseline:
  * software-pipelined finish: recip/normalize/accumulate for head k are
    emitted after head k+1's E+matmuls, so the in-order DVE/ACT queues never
    stall on PE completion.
  * 1/H folded into the rowsum ones-column (value H) -> haug copies need no
    scale -> graph 1's haug copy runs on idle GPSIMD, B/D exps merged to 2
    ACT ops (one [P,32] Exp each), h copies merged to 1.
  * head-0 C broadcast via a K=1 PE matmul (ones row x c_sb row) instead of
    the DRAM staging round trip; heads 1-7 still use staged stride-0 DMA.
  * output DMA reads the accumulator PSUM directly (no ACT copy).
  * x DMAs issued first; masks paced behind them on the other queue.
"""

from contextlib import ExitStack

import numpy as np

import concourse.bass as bass
import concourse.bacc as bacc
import concourse.tile as tile
from concourse import mybir
from concourse.bass_utils import run_bass_kernel_spmd

import concourse.dve_ops as dve_ops
from concourse.dve_spec import Spec, Src0, Src1, C0, C1, maxx, lower
from concourse.dve_uop import (
    DveOpSpec, UopConfig, InpSel, OutSel, OutPath,
    AluOp, AluInp, DelayInp, Trigger, ENABLE,
)

B, D, FIN, FOUT, H = 16, 512, 128, 128, 8
NCORES = 8
NB = B // NCORES          # graphs per core
P = 128                   # partitions
NCH = D // P              # 4 j-chunks / i-tiles
DELTA = -2.0              # global exp downshift (cancels in softmax)

EMASK_MODE = "2x"         # "2x" | "1x" (custom op without perf slot)

F32 = mybir.dt.float32
F16 = mybir.dt.float16

# consts f32 [P, 2]: delta col | C-bias col (-0.99*ab + DELTA)
CDL = 0
CBC = 1
CONST_COLS = 2

# constsH fp16 [P, 144]: W | Wa1 | Wa2
CW0, CW1 = 0, FOUT
CA1 = CW1 + H                           # Wa1 = W @ a1^T  [FIN, H]
CA2 = CA1 + H                           # Wa2 = W @ a2^T  [FIN, H]
CONSTH_COLS = CA2 + H

_NC_CACHE = {}
_EMASK = {}


# --------------------------------------------------------------------------
# custom DVE op: out = max(in0 * s0, s1) * in1, with 2x_1p table slot
# --------------------------------------------------------------------------

def _emask_ref(in0, in1, s0, s1, imm2):
    return np.maximum(in0.astype(np.float32) * s0, s1) * in1.astype(np.float32)


def _build_2x_uop():
    """2x_1p program: per cycle two packed fp16 elems (lo/hi) flow through
    6 compute blocks (mult, max, mult for each half) + 2 shuttle blocks.
    Mirrors the stock 2X_1P idiom: WR0_LO <- ALU_OUT, WR0_HI <- DELAY_0."""
    u = UopConfig()
    u.enable_input(InpSel.SRC_0, 1)      # d0 @blk0: cb lo
    u.enable_input(InpSel.CONST_0, 2)    # d1: D scalar
    u.enable_input(InpSel.CONST_1, 3)    # d2: B scalar
    u.enable_input(InpSel.SRC_1, 4)      # d3: mask lo
    u.enable_input(InpSel.SRC_0_HI, 5)   # d4: cb hi
    u.enable_input(InpSel.SRC_1_HI, 6)   # d5: mask hi
    dp = u.datapath_config
    dp[0].enable_alu(AluOp.MULTIPLY, AluInp.PREV_DELAY_0, AluInp.PREV_DELAY_1)
    dp[0].pass_through_delay(1, 2, 3, 4, 5)
    dp[1].enable_alu(AluOp.MAX, AluInp.PREV_ALU_OUT, AluInp.PREV_DELAY_2)
    dp[1].pass_through_delay(1, 2, 3, 4, 5)
    dp[2].enable_alu(AluOp.MULTIPLY, AluInp.PREV_ALU_OUT, AluInp.PREV_DELAY_3)
    dp[2].pass_through_delay(1, 2, 4, 5)
    dp[3].enable_alu(AluOp.MULTIPLY, AluInp.PREV_DELAY_4, AluInp.PREV_DELAY_1)
    dp[3].enable_delay_from_src(DelayInp.PREV_ALU_OUT, 0)   # capture LO
    dp[3].pass_through_delay(2, 5)
    dp[4].enable_alu(AluOp.MAX, AluInp.PREV_ALU_OUT, AluInp.PREV_DELAY_2)
    dp[4].pass_through_delay(0, 5)
    dp[5].enable_alu(AluOp.MULTIPLY, AluInp.PREV_ALU_OUT, AluInp.PREV_DELAY_5)
    dp[5].pass_through_delay(0)
    dp[6].enable_alu(AluOp.BYPASS, AluInp.PREV_DELAY_0, AluInp.PREV_DELAY_0)
    dp[6].enable_delay_from_src(DelayInp.PREV_ALU_OUT, 0)   # d0 <- HI
    dp[7].enable_alu(AluOp.BYPASS, AluInp.PREV_ALU_OUT, AluInp.PREV_ALU_OUT)
    dp[7].pass_through_delay(0)
    u.require_inp0 = ENABLE
    u.require_inp1 = ENABLE
    u.trigger = (Trigger.SRC_TENSOR_DONE, Trigger.NONE, Trigger.NONE)
    u.enable_output(OutSel.ALU_OUT, OutPath.WR0_LO)
    u.enable_output(OutSel.DELAY_0, OutPath.WR0_HI)
    return u


def _get_emask_op():
    perf = EMASK_MODE == "2x"
    name = "GAT_EMASK2X_ANT" if perf else "GAT_EMASK1X_ANT"
    if name in _EMASK:
        return _EMASK[name]
    spec = Spec(body=maxx(Src0 * C0, C1) * Src1, reference=_emask_ref)
    uops_1x = lower(spec, ver="v3")
    kw = dict(uops_2x=[_build_2x_uop()], perf_max=1) if perf else {}
    row = dve_ops._CUSTOM_DVE_ROW_BASE + len(dve_ops.OPS)
    spec_obj = DveOpSpec(name=name, opcode=row, uops=uops_1x, rd1_en=True, **kw)
    op = dve_ops.DveOp(name, spec, subdim=False,
                       uops_sha={"v3": spec_obj.sha("v3")})
    dve_ops.OPS.append(op)
    dve_ops.CUSTOM_DVE_SPECS[name] = spec
    dve_ops._SUB_OPCODE_FOR_NAME[name] = row
    dve_ops._COMPILE_CACHE[(name, "v3")] = spec_obj
    _EMASK[name] = op
    return op


def _emask(nc, out, in0, in1, s0, s1):
    # Inline of nc.vector._custom_dve that passes perf_max at construction
    # (add_instruction stores a copy, so post-hoc assignment is lost).
    import concourse.bass_isa as bass_isa

    op = _get_emask_op()
    v = nc.vector
    if op.name not in v.bass.m.ant_custom_dve_ops:
        v.bass.m.ant_custom_dve_ops = sorted(
            {*v.bass.m.ant_custom_dve_ops, op.name}
        )
    op.compile("v3")
    shape = bass_isa.CustomDveShape.TTSS
    isa_opcode = v.bass.isa.Opcode[
        f"NEURON_ISA_TPB_OPCODE_CUSTOM_DVE_ANT_{shape.slot()}"
    ].value
    ins = [
        v.lower_ap(in0, for_isa=True, opt=True),
        v.lower_ap(in1, for_isa=True, opt=True),
        v.lower_ap(s0, for_isa=True),
        v.lower_ap(s1, for_isa=True),
    ]
    outs = [v.lower_ap(out, for_isa=True, opt=True)]
    kw = {"perf_max": 1} if EMASK_MODE == "2x" else {}
    return v.add_instruction(
        bass_isa.InstCustomDveAnt(
            name=v.bass.get_next_instruction_name(),
            op_name=op.name, rd1_en=True, subdim=0, imm2=0.0,
            shape=shape, row=dve_ops.get_dve_sub_opcode(op.name),
            isa_opcode=isa_opcode, ins=ins, outs=outs, **kw,
        )
    )


# --------------------------------------------------------------------------
# kernel body
# --------------------------------------------------------------------------

def _build_bass():
    nc = bacc.Bacc("TRN2", debug=False, num_devices=NCORES)

    xT = nc.dram_tensor("xT", [NB, FIN, D], F16, kind="ExternalInput").ap()
    adjm = nc.dram_tensor("adjm", [NB, P, NCH * D], F16, kind="ExternalInput").ap()
    consts = nc.dram_tensor("consts", [P, CONST_COLS], F32, kind="ExternalInput").ap()
    constsH = nc.dram_tensor("constsH", [P, CONSTH_COLS], F16, kind="ExternalInput").ap()
    hsel = nc.dram_tensor("hsel", [NSEL, NSEL * P], F16, kind="ExternalInput").ap()
    ident = nc.dram_tensor("ident", [P, P], F16, kind="ExternalInput").ap()
    cd = nc.dram_tensor("cd", [NB, H, D], F16).ap()
    out = nc.dram_tensor("out", [NB, D, FOUT], F32, kind="ExternalOutput").ap()

    with tile.TileContext(nc) as tc, ExitStack() as ctx:
        _kernel_body(ctx, tc, out, xT, adjm, consts, constsH, hsel, ident, cd)
    nc.compile()
    return nc


def _kernel_body(ctx, tc, out, xT, adjm, consts, constsH, hsel, ident, cd):
    nc = tc.nc
    Copy = mybir.ActivationFunctionType.Copy
    Exp = mybir.ActivationFunctionType.Exp

    const = ctx.enter_context(tc.tile_pool(name="const", bufs=1))
    xpool = ctx.enter_context(tc.tile_pool(name="xpool", bufs=NB))
    mpool = ctx.enter_context(tc.tile_pool(name="mpool", bufs=NB))
    hpool = ctx.enter_context(tc.tile_pool(name="hpool", bufs=NB))
    bdpool = ctx.enter_context(tc.tile_pool(name="bdpool", bufs=2 * NB))
    crow = ctx.enter_context(tc.tile_pool(name="crow", bufs=NB))
    cbpool = ctx.enter_context(tc.tile_pool(name="cbpool", bufs=NB))
    epool = ctx.enter_context(tc.tile_pool(name="epool", bufs=4))
    unpool = ctx.enter_context(tc.tile_pool(name="unpool", bufs=3))
    rpool = ctx.enter_context(tc.tile_pool(name="rpool", bufs=4))
    aspool = ctx.enter_context(tc.tile_pool(name="aspool", bufs=NB))
    # PSUM: 2 setup + 4 agg (2 packed tiles x 2 bufs) + 2 accumulators = 8
    pset = ctx.enter_context(tc.tile_pool(name="pset", bufs=2, space="PSUM"))
    pout = ctx.enter_context(tc.tile_pool(name="pout", bufs=4, space="PSUM"))
    pacc = ctx.enter_context(tc.tile_pool(name="pacc", bufs=NB, space="PSUM"))

    # ---- phase A ---------------------------------------------------------
    # DGE descriptor generation costs ~0.6-1.1us PER dma_start, serially per
    # queue — so x0 (the gate for all setup compute) is the sole first issue
    # on the scalar queue; x1+masks flow on sync; consts + staged/derived
    # data ride gpsimd.
    xs0 = xpool.tile([FIN, D], F16, tag="x", name="x0")
    nc.scalar.dma_start(out=xs0, in_=xT[0])
    cst = const.tile([P, CONST_COLS], F32)
    nc.sync.dma_start(out=cst, in_=consts)
    csth = const.tile([P, CONSTH_COLS], F16)
    nc.sync.dma_start(out=csth, in_=constsH)
    xs1 = xpool.tile([FIN, D], F16, tag="x", name="x1")
    nc.sync.dma_start(out=xs1, in_=xT[1])
    x_sb = [xs0, xs1]
    m_sb = []
    for b in range(NB):
        ms = mpool.tile([P, NCH * D], F16, tag="m")
        nc.sync.dma_start(out=ms, in_=adjm[b])
        m_sb.append(ms)
    hsel_sb = const.tile([NSEL, NSEL * P], F16)
    nc.gpsimd.dma_start(out=hsel_sb, in_=hsel)
    I_sb = const.tile([P, P], F16)
    nc.gpsimd.dma_start(out=I_sb, in_=ident)
    W_sb = csth[:, CW0:CW1]
    Wa1_sb = csth[:, CW1:CA1]
    Wa2_sb = csth[:, CA1:CA2]
    def head_sel(hd):
        return hsel_sb[0:NSEL, hd * P:(hd + 1) * P]
    dl_col = cst[:, CDL:CDL + 1]
    cb_col = cst[:, CBC:CBC + 1]

    G = [dict(m_sb=m_sb[b]) for b in range(NB)]

    def setup_compute(b):
        """PE matmuls + exps for graph b. ab is folded into the C exp bias
        (cb_col), so s2 needs only fp16 matmuls. haug copies ride DVE (idle
        in lead-in). For b=0, heads 0-1's C broadcast comes from K=1 PE
        matmuls (no DRAM round trip); everything else via staged DMA on the
        gpsimd queue (the scalar queue is the busy ACT engine's sequencer)."""
        g = G[b]
        xs = x_sb[b]
        cb_all = cbpool.tile([P, H, D], F16, tag="cb")
        g["cb_all"] = cb_all

        # all x-gated matmuls first so the PE queue never stalls on ACT
        p_s1 = pset.tile([P, D], F32, tag="setup")
        nc.tensor.matmul(p_s1[0:H, :], Wa1_sb, xs[:], start=True, stop=True)
        p_s2 = pset.tile([P, NCH * H], F32, tag="setup")
        for c in range(NCH):
            nc.tensor.matmul(p_s2[:, c * H:(c + 1) * H],
                             xs[:, bass.ts(c, P)], Wa2_sb,
                             start=True, stop=True)
        p_h = pset.tile([P, NCH * FOUT], F32, tag="setup")
        for c in range(NCH):
            nc.tensor.matmul(p_h[:, c * FOUT:(c + 1) * FOUT],
                             xs[:, bass.ts(c, P)], W_sb, start=True, stop=True)

        # exps: B/D first (they gate the first E together with cb0)
        B_all = bdpool.tile([P, NCH * H], F32, tag=f"B{b}")
        nc.scalar.activation(B_all[:], p_s2[:], Exp, bias=dl_col)
        D_all = bdpool.tile([P, NCH * H], F32, tag=f"D{b}")
        nc.scalar.activation(D_all[:], p_s2[:], Exp, scale=0.01)
        g["B_all"], g["D_all"] = B_all, D_all
        c_sb = crow.tile([H, D], F16, tag="Crow")
        nc.scalar.activation(
            c_sb[:], p_s1[0:H, :], Exp, scale=-0.99, bias=cb_col[0:H, :]
        )
        nc.gpsimd.dma_start(out=cd[b], in_=c_sb[:])

        # graph 0 heads 0..NSEL-1: C broadcast via K=NSEL PE matmuls into the
        # (still idle) accumulator banks — no DRAM round-trip latency
        def pe_bcast(hd):
            p_cb = pacc.tile([P, D], F32, tag="acc", name=f"pcb{hd}")
            nc.tensor.matmul(p_cb[:], head_sel(hd), c_sb[0:NSEL, :],
                             start=True, stop=True)
            nc.scalar.activation(cb_all[:, hd, :], p_cb[:], Copy)

        # haug copy: DVE for graph 0 (idle in lead-in); ACT for graph 1 (the
        # DVE paces the steady loop by then, ACT has the slack)
        haug = hpool.tile([P, NCH, FOUT + 1], F16, tag="haug")
        hview = bass.AP(
            tensor=haug.tensor, offset=haug.offset,
            ap=[haug.ap[0], [FOUT + 1, NCH], [1, FOUT]],
        )
        if b == 0:
            nc.vector.tensor_copy(out=hview, in_=p_h[:])
        else:
            nc.scalar.activation(hview, p_h[:], Copy)
        for c in range(NCH):
            nc.vector.memset(haug[:, c, FOUT:FOUT + 1], float(H))
        g["haug"] = haug

        if b == 0:
            for hd in range(NSEL):
                pe_bcast(hd)

        # remaining heads' C broadcast via staged stride-0 DMA on the gpsimd
        # queue (the scalar queue is the busy ACT engine's sequencer)
        row0 = cd[b, 0]
        slices = [(NSEL, H)] if b == 0 else [(0, NSEL), (NSEL, H)]
        for lo, hi in slices:
            nc.gpsimd.dma_start(
                out=cb_all[:, lo:hi, :],
                in_=bass.AP(
                    tensor=cd.tensor, offset=row0.offset + lo * D,
                    ap=[[0, P], [D, hi - lo], row0.ap[-1]],
                ),
            )
        g["p_acc"] = pacc.tile([P, NCH * FOUT], F32, tag="acc", name=f"pacc{b}")

    setup_compute(0)

    # ---- main head loop, software-pipelined finish -----------------------
    def emit_E_and_agg(b, hd):
        g = G[b]
        cb = g["cb_all"][:, hd, :]
        E = epool.tile([P, NCH * D], F16, tag="E")
        for c in range(NCH):
            _emask(nc, E[:, bass.ts(c, D)], cb, g["m_sb"][:, bass.ts(c, D)],
                   g["D_all"][:, c * H + hd:c * H + hd + 1],
                   g["B_all"][:, c * H + hd:c * H + hd + 1])
        po2 = [pout.tile([P, 2, FOUT + 1], F32, tag="po", name=f"po{k}")
               for k in range(2)]
        p_os = [po2[t // 2][:, t % 2, :] for t in range(NCH)]
        for t in range(NCH):
            for c in range(NCH):
                nc.tensor.matmul(
                    p_os[t],
                    E[:, c * D + t * P: c * D + (t + 1) * P],
                    g["haug"][:, c, :],
                    start=(c == 0),
                    stop=(c == NCH - 1),
                )
        return po2, p_os

    def emit_finish(b, hd, po2, p_os):
        g = G[b]
        rall = rpool.tile([P, NCH], F32, tag="r")
        for k in range(2):
            nc.vector.reciprocal(rall[:, 2 * k:2 * k + 2], po2[k][:, :, FOUT])
        un = unpool.tile([P, NCH * FOUT], F16, tag="un")
        last = b == NB - 1 and hd == H - 1
        for t in range(NCH):
            if last and t % 2 == 1:
                # final drain: the now-idle DVE normalizes half the tiles in
                # parallel with ACT
                nc.vector.tensor_scalar_mul(
                    un[:, bass.ts(t, FOUT)], p_os[t][:, 0:FOUT],
                    rall[:, t:t + 1],
                )
            else:
                nc.scalar.activation(
                    un[:, bass.ts(t, FOUT)], p_os[t][:, 0:FOUT], Copy,
                    scale=rall[:, t:t + 1],
                )
        if hd < H - 1:
            nc.tensor.matmul(
                g["p_acc"][:], I_sb, un[:], start=(hd == 0), stop=False
            )
            return
        # last head: split accumulate/copy/store so output DMAs overlap the
        # remaining normalize+accumulate work (drain shortening). For the
        # final graph, quarter it across engines and DMA queues.
        acc_sb = aspool.tile([P, NCH * FOUT], F32, tag="accsb",
                             name=f"accsb{b}")
        ob = out[b]
        last = b == NB - 1
        nq = 2
        QW = NCH // nq
        queues = [nc.sync, nc.scalar, nc.gpsimd, nc.sync]
        for k in range(nq):
            cols = slice(k * QW * FOUT, (k + 1) * QW * FOUT)
            nc.tensor.matmul(
                g["p_acc"][:, cols], I_sb, un[:, cols], start=False, stop=True
            )
            if last and k % 2 == 1:
                nc.vector.tensor_copy(out=acc_sb[:, cols],
                                      in_=g["p_acc"][:, cols])
            else:
                nc.scalar.activation(acc_sb[:, cols], g["p_acc"][:, cols],
                                     Copy)
            (queues[k] if last else nc.sync).dma_start(
                out=bass.AP(
                    tensor=out.tensor,
                    offset=ob.offset + k * QW * P * FOUT,
                    ap=[[FOUT, P], [P * FOUT, QW], [1, FOUT]],
                ),
                in_=acc_sb[:, cols],
            )

    pend = None
    for b in range(NB):
        for hd in range(H):
            cur = (b, hd, *emit_E_and_agg(b, hd))
            if pend is not None:
                emit_finish(*pend)
            pend = cur
            if b == 0 and hd == 4:
                # graph 1 setup lands here, past the ramp: the ramp's first
                # iterations have no ACT slack for extra exps/copies.
                setup_compute(1)
    emit_finish(*pend)


# --------------------------------------------------------------------------
# host-side packing
# --------------------------------------------------------------------------

def _prep_core_inputs(input, adj, W, a_w, a_b, core):
    gs = slice(core * NB, (core + 1) * NB)
    x_c = np.asarray(input[gs], dtype=np.float32)     # [NB, D, FIN]
    adj_c = np.asarray(adj[gs])                       # [NB, D, D] int32
    xT = np.ascontiguousarray(x_c.transpose(0, 2, 1)).astype(np.float16)
    adjT = (adj_c.transpose(0, 2, 1) > 0)             # [NB, j, i]
    # [NB, j, i] -> [NB, p, c, i]  (j = c*128 + p)
    adjm = np.ascontiguousarray(
        adjT.reshape(NB, NCH, P, D).transpose(0, 2, 1, 3)
        .reshape(NB, P, NCH * D).astype(np.float16)
    )
    return {
        "xT": xT,
        "adjm": adjm,
        "consts": _pack_consts(W, a_w, a_b),
        "constsH": _pack_consts_h(W, a_w),
        "hsel": _pack_hsel(),
        "ident": np.eye(P, dtype=np.float16),
    }


def _pack_consts_h(W, a_w):
    W = np.asarray(W, dtype=np.float32)
    a_w = np.asarray(a_w, dtype=np.float32)
    c = np.zeros((P, CONSTH_COLS), dtype=np.float32)
    c[:, CW0:CW1] = W
    c[:, CW1:CA1] = W @ a_w[:, :FOUT].T               # Wa1 [FIN, H]
    c[:, CA1:CA2] = W @ a_w[:, FOUT:].T               # Wa2 [FIN, H]
    return c.astype(np.float16)


NSEL = 4  # heads broadcast via PE instead of the staged-DMA round trip


def _pack_hsel():
    # one-hot selector columns: bcast matmul lhsT [NSEL, P] picks c_sb row hd
    c = np.zeros((NSEL, NSEL * P), dtype=np.float16)
    for hd in range(NSEL):
        c[hd, hd * P:(hd + 1) * P] = 1.0
    return c


def _pack_consts(W, a_w, a_b):
    a_b = np.asarray(a_b, dtype=np.float32)
    c = np.zeros((P, CONST_COLS), dtype=np.float32)
    c[:, CDL] = DELTA
    # ab folded into the C exp: exp(ab_h) scales head h's whole E matrix and
    # cancels in the softmax row normalization, leaving a -0.99*ab_h shift on C.
    c[:H, CBC] = -0.99 * a_b + DELTA
    return c


def get_nc():
    if "nc" not in _NC_CACHE:
        _NC_CACHE["nc"] = _build_bass()
    return _NC_CACHE["nc"]


def run_on_device(in_maps, **kwargs):
    return run_bass_kernel_spmd(get_nc(), in_maps, list(range(NCORES)), **kwargs)


def kernel(input, adj, W, a_w, a_b):
    input = np.asarray(input, dtype=np.float32)
    adj = np.asarray(adj)

    in_maps = [
        _prep_core_inputs(input, adj, W, a_w, a_b, c) for c in range(NCORES)
    ]
    res = run_on_device(in_maps)
    outs = [res.results[c]["out"] for c in range(NCORES)]
    return np.concatenate(outs, axis=0).astype(np.float32)


if __name__ == "__main__":
    nc = get_nc()
    print("built ok")
